# revision 1
# baseline (speedup 1.0000x reference)
"""Blocksparse conv2d (3x3, stride 1, pad 1) on 8 Trainium2 NeuronCores.

Strategy
--------
Data-parallel over batch: 16 images -> 2 per core, identical SPMD program.

The mask zeroes whole 32x32 (cout, cin) channel blocks. The schedule is
specialized at build time from the *runtime* mask values (host inspects the
numpy mask inside kernel(), so any mask is handled correctly):
  - input-channel blocks that are entirely masked out are never loaded,
  - when only K_used <= 64 input channels survive, the channels are
    replicated into D = 128//K_used partition row-groups so that D
    row-tiled matmuls (different PE row groups) run concurrently.

Conv is lowered to 9 shifted matmuls accumulating in PSUM over a flat
row-padded image in SBUF (one zero row above and below, NO column
padding, so the x load is a single fully contiguous DMA per replica —
column-padded layouts need one DMA descriptor per image row, and SWDGE
descriptor emission at ~20ns each dominated the whole kernel).  Without
column zeros, the dw=0/dw=2 taps wrap across row boundaries, corrupting
output columns 0 and 127; those two columns are recomputed exactly with
12 small column-strided matmuls per image and overwritten during
copy-out.

Inputs are rounded to float32r during the load DMA (free SWDGE cast),
giving full-rate PE streaming at ~1e-4 matmul error.  Bias is fused into
the PSUM->SBUF copy on the scalar engine.
"""

import numpy as np
from contextlib import ExitStack

import concourse.bass as bass
import concourse.tile as tile
from concourse import mybir, bacc
from concourse import bass_utils
from concourse.masks import make_identity

# Problem shape (hardcoded per contract)
B, CIN, COUT, H, W = 16, 128, 128, 128, 128
KH, KW = 3, 3
BLK = 32
NCORES = 8
BPC = B // NCORES            # images per core

IMG0 = 1 + W                 # flat offset of image row 0 (1 slack el + top pad row)
LFLAT = IMG0 + H * W + W + 1  # + bottom pad row + 1 slack el
RPW = 3                      # output rows per PSUM window
NWIN = (H + RPW - 1) // RPW  # 43 windows (42 full + one 2-row)
CHUNK_WIN = 8                # windows staged per output DMA (24 rows)
GROUP_WIN = 4                # windows per matmul group (tap-outer ordering)

_cache = {}


def _build(n_ib, D, mm_order="t_outer", repeat=1, mm_dtype="f32r", taps=9, x_mode="swdge_cast"):
    """Build + compile the per-core SPMD program.

    n_ib: number of surviving 32-channel input blocks (1..4)
    D:    partition replication factor (1, 2 or 4); D*K_used <= 128
    repeat/taps/mm_*: benchmarking knobs (repeat>1 wraps compute in For_i)
    """
    K_used = BLK * n_ib
    DK = D * K_used
    assert DK <= 128

    nc = bacc.Bacc("TRN2", target_bir_lowering=False, debug=False)
    f32 = mybir.dt.float32
    f32r = mybir.dt.float32r if mm_dtype == "f32r" else mybir.dt.bfloat16

    # channels arrive pre-replicated D times from the host (a step-0
    # replicated-source DMA measured ~7x below line rate)
    x_in = nc.dram_tensor("x", [BPC, DK, H, W], f32, kind="ExternalInput").ap()
    w_in = nc.dram_tensor("w", [COUT, K_used, KH, KW], f32, kind="ExternalInput").ap()
    m_in = nc.dram_tensor("m", [COUT, K_used, KH, KW], f32, kind="ExternalInput").ap()
    b_in = nc.dram_tensor("bias", [COUT], f32, kind="ExternalInput").ap()
    y_out = nc.dram_tensor("y", [BPC, COUT, H, W], f32, kind="ExternalOutput").ap()

    with tile.TileContext(nc) as tc:
        with ExitStack() as ctx:
            singles = ctx.enter_context(tc.tile_pool(name="singles", bufs=1))
            stage_pool = ctx.enter_context(tc.tile_pool(name="ystage", bufs=3))
            psum_pool = ctx.enter_context(
                tc.tile_pool(name="psum", bufs=6, space="PSUM")
            )
            epsum_pool = ctx.enter_context(
                tc.tile_pool(name="epsum", bufs=2, space="PSUM")
            )

            # ---- one-time setup: weights, bias, identity ------------------
            ident = singles.tile([128, 128], f32, name="ident")
            make_identity(nc, ident)

            bias_sb = singles.tile([COUT, 1], f32, name="bias_sb")
            nc.sync.dma_start(out=bias_sb, in_=b_in.unsqueeze(1))

            w_raw = singles.tile([COUT, K_used, KH * KW], f32, name="w_raw")
            m_raw = singles.tile([COUT, K_used, KH * KW], f32, name="m_raw")
            nc.sync.dma_start(out=w_raw, in_=w_in.rearrange("o c kh kw -> o c (kh kw)"))
            nc.sync.dma_start(out=m_raw, in_=m_in.rearrange("o c kh kw -> o c (kh kw)"))
            nc.vector.tensor_tensor(
                out=w_raw, in0=w_raw, in1=m_raw, op=mybir.AluOpType.mult
            )

            # wT[d*K_used + c, t, o] = (w*m)[o, c, tap t], replicated D times
            # along partitions so each PE row-group has its own copy.
            wm2 = singles.tile([COUT, 128], f32, name="wm2")
            wT = singles.tile([128, KH * KW, COUT], f32r, name="wT")
            for t in range(KH * KW):
                for s in range(D):
                    nc.vector.tensor_copy(
                        out=wm2[:, s * K_used : (s + 1) * K_used],
                        in_=w_raw[:, :, t],
                    )
                ps_t = psum_pool.tile([128, 512], f32, tag="ps", name=f"ps_t{t}")
                nc.tensor.transpose(ps_t[:DK, :COUT], wm2[:, :DK], ident)
                nc.vector.tensor_copy(out=wT[:DK, t, :], in_=ps_t[:DK, :COUT])

            # ---- persistent flat row-padded x buffers ---------------------
            # memset doesn't support float32r; pad zeros come from DVE copies
            # (fp32 -> f32r cast) out of a zeroed fp32 scratch.
            zsc = singles.tile([128, IMG0], f32, name="zsc")
            nc.gpsimd.memset(zsc, 0.0)
            xbufs = []
            for i in range(BPC):
                xb = singles.tile([128, LFLAT], f32r, name=f"xbuf{i}")
                nc.vector.tensor_copy(out=xb[:DK, :IMG0], in_=zsc[:DK, :])
                nc.vector.tensor_copy(
                    out=xb[:DK, IMG0 + H * W :], in_=zsc[:DK, : W + 1]
                )
                xbufs.append(xb)

            # ---- per-image pipeline --------------------------------------
            if repeat > 1:
                loop_cm = tc.For_i(0, repeat, 1)
                loop_cm.__enter__()
            for b in range(BPC):
                xb = xbufs[b]
                if taps < 0:
                    nc.vector.tensor_copy(out=xb[:DK, 0:8], in_=zsc[:DK, 0:8])
                    continue
                src = x_in[b].rearrange("c h w -> c (h w)")  # [DK, H*W]
                dst = xb[:DK, IMG0 : IMG0 + H * W]
                if x_mode == "swdge_cast":
                    # fp32 -> f32r rounding happens inside the SWDGE DMA
                    nc.gpsimd.dma_start(out=dst, in_=src)
                else:  # hwdge_f32: timing probe only (wrong dtype for MMs)
                    nc.sync.dma_start(out=dst.bitcast(mybir.dt.float32), in_=src)
                if taps == 0:
                    continue

                # -- exact edge columns (j=0 and j=127) into edge psum ------
                # main-path taps wrap across rows there; recompute from
                # column-strided views and overwrite during copy-out.
                edge_ps = epsum_pool.tile([128, 512], f32, tag="eps", name=f"eps{b}")
                # NOTE: keep each edge's 6 accumulating matmuls contiguous —
                # start=True resets the whole bank's has_written bits, so
                # interleaving the two groups corrupts the first one.
                edge_mms = []  # (psum_off, t, col)
                for dh in range(KH):
                    for dwsel in (1, 2):  # left edge j=0
                        edge_mms.append((0, dh * 3 + dwsel, dwsel - 1))
                for dh in range(KH):
                    for dwsel in (0, 1):  # right edge j=127
                        edge_mms.append((128, dh * 3 + dwsel, 126 + dwsel))
                n_per = {0: 0, 128: 0}
                for eoff, _, _ in edge_mms:
                    n_per[eoff] += 1
                seen = {0: 0, 128: 0}
                for eoff, t, col in edge_mms:
                    dh = t // 3
                    o = IMG0 + (dh - 1) * W + col
                    v = xb[0:K_used, o : o + 1]
                    rhs = bass.AP(
                        tensor=v.tensor,
                        offset=v.offset,
                        ap=[list(v.ap[0]), [W, H]],
                    )
                    nc.tensor.matmul(
                        edge_ps[:, eoff : eoff + H],
                        wT[0:K_used, t, :],
                        rhs,
                        start=(seen[eoff] == 0),
                        stop=(seen[eoff] == n_per[eoff] - 1),
                    )
                    seen[eoff] += 1

                win_rows = [RPW * w for w in range(NWIN)]
                for c0 in range(0, NWIN, CHUNK_WIN):
                    wins = list(range(c0, min(c0 + CHUNK_WIN, NWIN)))
                    chunk_r0 = RPW * c0
                    chunk_nr = min(RPW * len(wins), H - chunk_r0)
                    stage = stage_pool.tile(
                        [COUT, RPW * CHUNK_WIN, W], f32, tag="stage", name=f"st{b}_{c0}"
                    )

                    for g0 in range(0, len(wins), GROUP_WIN):
                        group = wins[g0 : g0 + GROUP_WIN]
                        ps = {}
                        for w in group:
                            ps[w] = psum_pool.tile(
                                [128, 512], f32, tag="ps", name=f"ps{b}_{w}"
                            )
                        if mm_order == "t_outer":
                            mm_seq = [(t, w) for t in range(taps) for w in group]
                        else:
                            mm_seq = [(t, w) for w in group for t in range(taps)]
                        for t, w in mm_seq:
                            dh, dw = divmod(t, 3)
                            s = w % D
                            r0 = win_rows[w]
                            nrows = min(RPW, H - r0)
                            N = nrows * W
                            q0 = IMG0 + (r0 + dh - 1) * W + (dw - 1)
                            nc.tensor.matmul(
                                ps[w][:, :N],
                                wT[s * K_used : (s + 1) * K_used, t, :],
                                xb[s * K_used : (s + 1) * K_used, q0 : q0 + N],
                                start=(t == 0),
                                stop=(t == taps - 1),
                            )
                        # copy-out with fused bias
                        for w in group:
                            r0 = win_rows[w]
                            nrows = min(RPW, H - r0)
                            ps_v = ps[w][:, : nrows * W].rearrange(
                                "p (r s) -> p r s", s=W
                            )
                            nc.scalar.activation(
                                out=stage[:, r0 - chunk_r0 : r0 - chunk_r0 + nrows, :],
                                in_=ps_v,
                                func=mybir.ActivationFunctionType.Identity,
                                bias=bias_sb,
                                scale=1.0,
                            )
                    # overwrite the two corrupted edge columns
                    nc.scalar.activation(
                        out=stage[:, :chunk_nr, 0:1],
                        in_=edge_ps[:, chunk_r0 : chunk_r0 + chunk_nr].unsqueeze(2),
                        func=mybir.ActivationFunctionType.Identity,
                        bias=bias_sb,
                        scale=1.0,
                    )
                    nc.scalar.activation(
                        out=stage[:, :chunk_nr, W - 1 : W],
                        in_=edge_ps[
                            :, 128 + chunk_r0 : 128 + chunk_r0 + chunk_nr
                        ].unsqueeze(2),
                        func=mybir.ActivationFunctionType.Identity,
                        bias=bias_sb,
                        scale=1.0,
                    )

                    nc.sync.dma_start(
                        out=y_out[b][:, chunk_r0 : chunk_r0 + chunk_nr, :],
                        in_=stage[:, :chunk_nr, :],
                    )

            if repeat > 1:
                loop_cm.__exit__(None, None, None)

    nc.compile()
    return nc


def kernel(x, weight, bias, mask):
    x = np.ascontiguousarray(np.asarray(x, dtype=np.float32))
    weight = np.ascontiguousarray(np.asarray(weight, dtype=np.float32))
    bias = np.ascontiguousarray(np.asarray(bias, dtype=np.float32))
    mask = np.ascontiguousarray(np.asarray(mask, dtype=np.float32))

    # --- host-side schedule specialization from the runtime mask ----------
    wm = weight * mask
    blk_any = (
        np.abs(wm).reshape(COUT, CIN // BLK, BLK, KH, KW).sum(axis=(0, 2, 3, 4)) > 0
    )
    used_ibs = [ib for ib in range(CIN // BLK) if blk_any[ib]]
    if not used_ibs:
        used_ibs = [0]
    n_ib = len(used_ibs)
    K_used = BLK * n_ib
    D = 128 // K_used if K_used <= 64 else 1

    used_ch = np.concatenate(
        [np.arange(ib * BLK, (ib + 1) * BLK) for ib in used_ibs]
    )

    key = (n_ib, D)
    if key not in _cache:
        _cache[key] = _build(n_ib, D)
    nc = _cache[key]

    w_slice = np.ascontiguousarray(weight[:, used_ch])
    m_slice = np.ascontiguousarray(mask[:, used_ch])
    rep_ch = np.concatenate([used_ch] * D)  # host-side replica duplication
    in_maps = []
    for core in range(NCORES):
        xs = np.ascontiguousarray(x[core * BPC : (core + 1) * BPC][:, rep_ch])
        in_maps.append({"x": xs, "w": w_slice, "m": m_slice, "bias": bias})

    global _last_in_maps
    _last_in_maps = in_maps

    res = bass_utils.run_bass_kernel_spmd(nc, in_maps, core_ids=list(range(NCORES)))
    y = np.concatenate([res.results[c]["y"] for c in range(NCORES)], axis=0)
    return y


_last_in_maps = None



# revision 2
# speedup vs baseline: 1.8819x; 1.8819x over previous
"""Blocksparse conv2d (3x3, stride 1, pad 1) on 8 Trainium2 NeuronCores.

Strategy (v2)
-------------
Data-parallel over batch: 16 images -> 2 per core, identical SPMD program.

The mask zeroes whole 32x32 (cout, cin) channel blocks; the host inspects
the runtime mask and keeps only input-channel blocks that survive
(seed-42 mask: channels 64..127, K_used=64). Host-side prep (all cheap,
weight-sized or a single pass over x):
  - w*mask, slice to used channels, transpose to the PE lhsT layout
    [K_used, 9, COUT] and cast to bf16 (fp32r matmuls run LOW_HIGH
    2-pass on TRN2; bf16 streams 1 row/cycle),
  - x sliced to used channels, zero-padded to (H+2)x(W+2) and cast to
    bf16. The pad border makes every conv tap a plain strided view of
    the flat image -- no edge-column recompute, and x lands in SBUF as
    a handful of large contiguous DMAs.

On-chip layout ("halves", when 2*K_used <= 128): image 0 lives in
partitions [0, K), image 1 in [K, 2K), each with its own weight copy
(host-replicated). The two images' matmuls interleave per tap, so two
row-group streams run concurrently on the PE (2 cols/cycle aggregate).
x arrives in row-bands via SWDGE (gpsimd) so the first matmuls start a
few us in, overlapping the rest of the load; weights/bias load first on
uncontended queues (baseline lost 60us to weight DMAs starved behind
the bulk x transfer). PSUM holds 4 windows x 2 images (8 banks);
windows complete tap-by-tap (w-outer) so the scalar-engine bias+copy
drains each window while the next streams. Output chunks (24 rows) DMA
from staging on the sync HWDGE queue.
"""

import numpy as np
import ml_dtypes
from contextlib import ExitStack

import concourse.bass as bass
import concourse.tile as tile
from concourse import mybir, bacc
from concourse import bass_utils

# Problem shape (hardcoded per contract)
B, CIN, COUT, H, W = 16, 128, 128, 128, 128
KH, KW = 3, 3
BLK = 32
NCORES = 8
BPC = B // NCORES            # images per core

HP, WP = H + 2, W + 2        # host-padded image
LF = HP * WP                 # flat padded image length (16900)
RPW = 3                      # output rows per PSUM window
NWIN = (H + RPW - 1) // RPW  # 43 windows (42 full + one 2-row)
CHUNK_WIN = 8                # windows staged per output DMA (24 rows)

# x row-bands (in padded rows): first band small for a fast pipeline start
BANDS = [(0, 16), (16, 32), (48, 32), (80, 32), (112, 18)]

_cache = {}


def _build(K_used, halves):
    """Build + compile the per-core SPMD program.

    K_used: number of surviving input channels (multiple of 32)
    halves: both images packed into one 128-partition buffer at bases
            (0, K_used) for concurrent PE row-group streams
    """
    P = 2 * K_used if halves else K_used
    assert P <= 128

    nc = bacc.Bacc("TRN2", target_bir_lowering=False, debug=False)
    f32 = mybir.dt.float32
    bf16 = mybir.dt.bfloat16

    x_in = nc.dram_tensor("xp", [BPC, K_used, HP, WP], bf16, kind="ExternalInput").ap()
    wt_in = nc.dram_tensor("wt", [P, KH * KW, COUT], bf16, kind="ExternalInput").ap()
    b_in = nc.dram_tensor("bias2d", [COUT, 1], f32, kind="ExternalInput").ap()
    y_out = nc.dram_tensor("y", [BPC, COUT, H, W], f32, kind="ExternalOutput").ap()

    with tile.TileContext(nc) as tc:
        with ExitStack() as ctx:
            singles = ctx.enter_context(tc.tile_pool(name="singles", bufs=1))
            stage_pool = ctx.enter_context(tc.tile_pool(name="ystage", bufs=4))
            psum_pool = ctx.enter_context(
                tc.tile_pool(name="psum", bufs=8, space="PSUM")
            )

            # ---- small loads first: weights on SWDGE, bias on HWDGE -------
            wT = singles.tile([P, KH * KW, COUT], bf16, name="wT")
            nc.gpsimd.dma_start(out=wT, in_=wt_in)

            bias_sb = singles.tile([COUT, 1], f32, name="bias_sb")
            nc.sync.dma_start(out=bias_sb, in_=b_in)

            # ---- x: flat padded images, loaded in row-bands (SWDGE) -------
            if halves:
                xb0 = singles.tile([128, LF], bf16, name="xbuf")
                xbufs = [xb0, xb0]
                pbase = [0, K_used]
            else:
                xbufs = [
                    singles.tile([K_used, LF], bf16, name=f"xbuf{i}")
                    for i in range(BPC)
                ]
                pbase = [0] * BPC
            for r0, nr in BANDS:
                for b in range(BPC):
                    src = x_in[b].rearrange("c h w -> c (h w)")
                    nc.gpsimd.dma_start(
                        out=xbufs[b][pbase[b] : pbase[b] + K_used, r0 * WP : (r0 + nr) * WP],
                        in_=src[:, r0 * WP : (r0 + nr) * WP],
                    )

            # ---- main loop: windows of RPW output rows --------------------
            for c0 in range(0, NWIN, CHUNK_WIN):
                wins = list(range(c0, min(c0 + CHUNK_WIN, NWIN)))
                chunk_r0 = RPW * c0
                chunk_nr = min(RPW * len(wins), H - chunk_r0)
                stages = [
                    stage_pool.tile(
                        [COUT, RPW * CHUNK_WIN * W], f32, tag=f"st{b}", name=f"st{b}_{c0}"
                    )
                    for b in range(BPC)
                ]

                for w in wins:
                    r0 = RPW * w
                    nrows = min(RPW, H - r0)
                    N = nrows * W
                    ps = [
                        psum_pool.tile([128, 512], f32, tag="ps", name=f"ps{b}_{w}")
                        for b in range(BPC)
                    ]
                    for t in range(KH * KW):
                        dh, dw = divmod(t, 3)
                        off = (r0 + dh) * WP + dw
                        for b in range(BPC):
                            v = xbufs[b][pbase[b] : pbase[b] + K_used, off : off + 1]
                            rhs = bass.AP(
                                tensor=v.tensor,
                                offset=v.offset,
                                ap=[list(v.ap[0]), [WP, nrows], [1, W]],
                            )
                            nc.tensor.matmul(
                                ps[b][:, :N],
                                wT[pbase[b] : pbase[b] + K_used, t, :],
                                rhs,
                                start=(t == 0),
                                stop=(t == KH * KW - 1),
                            )
                    # per-window bias + PSUM->SBUF drain on the scalar engine
                    so = (w - c0) * RPW * W
                    for b in range(BPC):
                        nc.scalar.activation(
                            out=stages[b][:, so : so + N],
                            in_=ps[b][:, :N],
                            func=mybir.ActivationFunctionType.Identity,
                            bias=bias_sb,
                            scale=1.0,
                        )

                for b in range(BPC):
                    nc.sync.dma_start(
                        out=y_out[b][:, chunk_r0 : chunk_r0 + chunk_nr, :],
                        in_=stages[b][:, : chunk_nr * W].rearrange(
                            "p (r s) -> p r s", s=W
                        ),
                    )

    nc.compile()
    return nc


def kernel(x, weight, bias, mask):
    x = np.ascontiguousarray(np.asarray(x, dtype=np.float32))
    weight = np.ascontiguousarray(np.asarray(weight, dtype=np.float32))
    bias = np.ascontiguousarray(np.asarray(bias, dtype=np.float32))
    mask = np.ascontiguousarray(np.asarray(mask, dtype=np.float32))
    bf16 = ml_dtypes.bfloat16

    # --- host-side schedule specialization from the runtime mask ----------
    wm = weight * mask
    blk_any = (
        np.abs(wm).reshape(COUT, CIN // BLK, BLK, KH, KW).sum(axis=(0, 2, 3, 4)) > 0
    )
    used_ibs = [ib for ib in range(CIN // BLK) if blk_any[ib]]
    if not used_ibs:
        used_ibs = [0]
    K_used = BLK * len(used_ibs)
    halves = 2 * K_used <= 128

    used_ch = np.concatenate(
        [np.arange(ib * BLK, (ib + 1) * BLK) for ib in used_ibs]
    )

    key = (K_used, halves)
    if key not in _cache:
        _cache[key] = _build(K_used, halves)
    nc = _cache[key]

    # lhsT layout: wt[c, t, o] = (w*m)[o, used_ch[c], t], replicated per
    # partition half so each image's row group has its own copy
    wt = wm[:, used_ch].reshape(COUT, K_used, KH * KW).transpose(1, 2, 0)
    if halves:
        wt = np.concatenate([wt, wt], axis=0)
    wt = np.ascontiguousarray(wt.astype(bf16))
    bias2d = np.ascontiguousarray(bias[:, None])

    in_maps = []
    for core in range(NCORES):
        xs = x[core * BPC : (core + 1) * BPC][:, used_ch]
        xp = np.zeros((BPC, K_used, HP, WP), dtype=bf16)
        xp[:, :, 1 : H + 1, 1 : W + 1] = xs.astype(bf16)
        in_maps.append({"xp": xp, "wt": wt, "bias2d": bias2d})

    global _last_in_maps
    _last_in_maps = in_maps

    res = bass_utils.run_bass_kernel_spmd(nc, in_maps, core_ids=list(range(NCORES)))
    y = np.concatenate([res.results[c]["y"] for c in range(NCORES)], axis=0)
    return y


_last_in_maps = None


# revision 6
# speedup vs baseline: 1.9690x; 1.0463x over previous
"""Blocksparse conv2d (3x3, stride 1, pad 1) on 8 Trainium2 NeuronCores.

Strategy (v2)
-------------
Data-parallel over batch: 16 images -> 2 per core, identical SPMD program.

The mask zeroes whole 32x32 (cout, cin) channel blocks; the host inspects
the runtime mask and keeps only input-channel blocks that survive
(seed-42 mask: channels 64..127, K_used=64). Host-side prep (all cheap,
weight-sized or a single pass over x):
  - w*mask, slice to used channels, transpose to the PE lhsT layout
    [K_used, 9, COUT] and cast to bf16 (fp32r matmuls run LOW_HIGH
    2-pass on TRN2; bf16 streams 1 row/cycle),
  - x sliced to used channels, zero-padded to (H+2)x(W+2) and cast to
    bf16. The pad border makes every conv tap a plain strided view of
    the flat image -- no edge-column recompute, and x lands in SBUF as
    a handful of large contiguous DMAs.

On-chip layout ("halves", when 2*K_used <= 128): image 0 lives in
partitions [0, K), image 1 in [K, 2K), each with its own weight copy
(host-replicated). The two images' matmuls interleave per tap, so two
row-group streams run concurrently on the PE (2 cols/cycle aggregate).
x arrives in row-bands via SWDGE (gpsimd) so the first matmuls start a
few us in, overlapping the rest of the load; weights/bias load first on
uncontended queues (baseline lost 60us to weight DMAs starved behind
the bulk x transfer). PSUM holds 4 windows x 2 images (8 banks);
windows complete tap-by-tap (w-outer) so the scalar-engine bias+copy
drains each window while the next streams. Output chunks (24 rows) DMA
from staging on the sync HWDGE queue.
"""

import numpy as np
import ml_dtypes
from contextlib import ExitStack

import concourse.bass as bass
import concourse.tile as tile
from concourse import mybir, bacc
from concourse import bass_utils

# Problem shape (hardcoded per contract)
B, CIN, COUT, H, W = 16, 128, 128, 128, 128
KH, KW = 3, 3
BLK = 32
NCORES = 8
BPC = B // NCORES            # images per core

HP, WP = H + 2, W + 2        # host-padded image
LF = HP * WP                 # flat padded image length (16900)
RPW = 4                      # output rows per PSUM window (N=512 = full bank)
NWIN = H // RPW              # 32 uniform windows
CHUNK_WIN = 2                # windows staged per output DMA (8 rows)
NWARM = 10                   # PE warm-up matmuls (ramp DVFS before x lands)

# x row-bands (in padded rows): first band small for a fast pipeline start
BANDS = [(0, 16), (16, 32), (48, 32), (80, 32), (112, 18)]

_cache = {}


def _build(K_used, halves):
    """Build + compile the per-core SPMD program.

    K_used: number of surviving input channels (multiple of 32)
    halves: both images packed into one 128-partition buffer at bases
            (0, K_used) for concurrent PE row-group streams
    """
    P = 2 * K_used if halves else K_used
    assert P <= 128

    nc = bacc.Bacc("TRN2", target_bir_lowering=False, debug=False)
    f32 = mybir.dt.float32
    bf16 = mybir.dt.bfloat16

    x_in = nc.dram_tensor("xp", [BPC, K_used, HP, WP], bf16, kind="ExternalInput").ap()
    wt_in = nc.dram_tensor("wt", [P, KH * KW, COUT], bf16, kind="ExternalInput").ap()
    b_in = nc.dram_tensor("bias2d", [COUT, 1], f32, kind="ExternalInput").ap()
    y_out = nc.dram_tensor("y", [BPC, COUT, H, W], f32, kind="ExternalOutput").ap()

    with tile.TileContext(nc) as tc:
        with ExitStack() as ctx:
            singles = ctx.enter_context(tc.tile_pool(name="singles", bufs=1))
            stage_pool = ctx.enter_context(tc.tile_pool(name="ystage", bufs=6))
            psum_pool = ctx.enter_context(
                tc.tile_pool(name="psum", bufs=8, space="PSUM")
            )

            # ---- small loads first: weights+bias on the sync HWDGE queue
            # (uncontended; x bands own the gpsimd SWDGE queue) ------------
            wT = singles.tile([P, KH * KW, COUT], bf16, name="wT")
            nc.sync.dma_start(out=wT, in_=wt_in)

            bias_sb = singles.tile([COUT, 1], f32, name="bias_sb")
            nc.sync.dma_start(out=bias_sb, in_=b_in)

            # ---- PE warm-up: zero matmuls with no DMA dependencies keep
            # the tensor engine busy through the p-state ramp while the
            # first x bands are still in flight --------------------------
            warm = singles.tile([128, 512], bf16, name="warm")
            nc.gpsimd.memset(warm, 0.0)
            warm_ps = psum_pool.tile([128, 512], f32, tag="ps", name="warm_ps")
            for i in range(NWARM):
                nc.tensor.matmul(
                    warm_ps,
                    warm[0:64, 0:128],
                    warm[0:64, :],
                    start=True,
                    stop=True,
                )

            # ---- x: flat padded images, loaded in row-bands (SWDGE) -------
            if halves:
                xb0 = singles.tile([128, LF], bf16, name="xbuf")
                xbufs = [xb0, xb0]
                pbase = [0, K_used]
            else:
                xbufs = [
                    singles.tile([K_used, LF], bf16, name=f"xbuf{i}")
                    for i in range(BPC)
                ]
                pbase = [0] * BPC
            for r0, nr in BANDS:
                for b in range(BPC):
                    src = x_in[b].rearrange("c h w -> c (h w)")
                    nc.gpsimd.dma_start(
                        out=xbufs[b][pbase[b] : pbase[b] + K_used, r0 * WP : (r0 + nr) * WP],
                        in_=src[:, r0 * WP : (r0 + nr) * WP],
                    )

            # ---- main loop: windows of RPW output rows --------------------
            for c0 in range(0, NWIN, CHUNK_WIN):
                wins = list(range(c0, min(c0 + CHUNK_WIN, NWIN)))
                chunk_r0 = RPW * c0
                chunk_nr = min(RPW * len(wins), H - chunk_r0)
                stages = [
                    stage_pool.tile(
                        [COUT, RPW * CHUNK_WIN * W], f32, tag=f"st{b}", name=f"st{b}_{c0}"
                    )
                    for b in range(BPC)
                ]

                for w in wins:
                    r0 = RPW * w
                    nrows = min(RPW, H - r0)
                    N = nrows * W
                    ps = [
                        psum_pool.tile([128, 512], f32, tag="ps", name=f"ps{b}_{w}")
                        for b in range(BPC)
                    ]
                    for t in range(KH * KW):
                        dh, dw = divmod(t, 3)
                        off = (r0 + dh) * WP + dw
                        for b in range(BPC):
                            v = xbufs[b][pbase[b] : pbase[b] + K_used, off : off + 1]
                            rhs = bass.AP(
                                tensor=v.tensor,
                                offset=v.offset,
                                ap=[list(v.ap[0]), [WP, nrows], [1, W]],
                            )
                            nc.tensor.matmul(
                                ps[b][:, :N],
                                wT[pbase[b] : pbase[b] + K_used, t, :],
                                rhs,
                                start=(t == 0),
                                stop=(t == KH * KW - 1),
                            )
                    # per-window bias + PSUM->SBUF drain on the scalar engine
                    so = (w - c0) * RPW * W
                    for b in range(BPC):
                        nc.scalar.activation(
                            out=stages[b][:, so : so + N],
                            in_=ps[b][:, :N],
                            func=mybir.ActivationFunctionType.Identity,
                            bias=bias_sb,
                            scale=1.0,
                        )

                for b in range(BPC):
                    nc.sync.dma_start(
                        out=y_out[b][:, chunk_r0 : chunk_r0 + chunk_nr, :],
                        in_=stages[b][:, : chunk_nr * W].rearrange(
                            "p (r s) -> p r s", s=W
                        ),
                    )

    nc.compile()
    return nc


def kernel(x, weight, bias, mask):
    x = np.ascontiguousarray(np.asarray(x, dtype=np.float32))
    weight = np.ascontiguousarray(np.asarray(weight, dtype=np.float32))
    bias = np.ascontiguousarray(np.asarray(bias, dtype=np.float32))
    mask = np.ascontiguousarray(np.asarray(mask, dtype=np.float32))
    bf16 = ml_dtypes.bfloat16

    # --- host-side schedule specialization from the runtime mask ----------
    wm = weight * mask
    blk_any = (
        np.abs(wm).reshape(COUT, CIN // BLK, BLK, KH, KW).sum(axis=(0, 2, 3, 4)) > 0
    )
    used_ibs = [ib for ib in range(CIN // BLK) if blk_any[ib]]
    if not used_ibs:
        used_ibs = [0]
    K_used = BLK * len(used_ibs)
    halves = 2 * K_used <= 128

    used_ch = np.concatenate(
        [np.arange(ib * BLK, (ib + 1) * BLK) for ib in used_ibs]
    )

    key = (K_used, halves)
    if key not in _cache:
        _cache[key] = _build(K_used, halves)
    nc = _cache[key]

    # lhsT layout: wt[c, t, o] = (w*m)[o, used_ch[c], t], replicated per
    # partition half so each image's row group has its own copy
    wt = wm[:, used_ch].reshape(COUT, K_used, KH * KW).transpose(1, 2, 0)
    if halves:
        wt = np.concatenate([wt, wt], axis=0)
    wt = np.ascontiguousarray(wt.astype(bf16))
    bias2d = np.ascontiguousarray(bias[:, None])

    in_maps = []
    for core in range(NCORES):
        xs = x[core * BPC : (core + 1) * BPC][:, used_ch]
        xp = np.zeros((BPC, K_used, HP, WP), dtype=bf16)
        xp[:, :, 1 : H + 1, 1 : W + 1] = xs.astype(bf16)
        in_maps.append({"xp": xp, "wt": wt, "bias2d": bias2d})

    global _last_in_maps
    _last_in_maps = in_maps

    res = bass_utils.run_bass_kernel_spmd(nc, in_maps, core_ids=list(range(NCORES)))
    y = np.concatenate([res.results[c]["y"] for c in range(NCORES)], axis=0)
    return y


_last_in_maps = None


# revision 10
# speedup vs baseline: 2.0000x; 1.0157x over previous
"""Blocksparse conv2d (3x3, stride 1, pad 1) on 8 Trainium2 NeuronCores.

Strategy (v2)
-------------
Data-parallel over batch: 16 images -> 2 per core, identical SPMD program.

The mask zeroes whole 32x32 (cout, cin) channel blocks; the host inspects
the runtime mask and keeps only input-channel blocks that survive
(seed-42 mask: channels 64..127, K_used=64). Host-side prep (all cheap,
weight-sized or a single pass over x):
  - w*mask, slice to used channels, transpose to the PE lhsT layout
    [K_used, 9, COUT] and cast to bf16 (fp32r matmuls run LOW_HIGH
    2-pass on TRN2; bf16 streams 1 row/cycle),
  - x sliced to used channels, zero-padded to (H+2)x(W+2) and cast to
    bf16. The pad border makes every conv tap a plain strided view of
    the flat image -- no edge-column recompute, and x lands in SBUF as
    a handful of large contiguous DMAs.

On-chip layout ("halves", when 2*K_used <= 128): image 0 lives in
partitions [0, K), image 1 in [K, 2K), each with its own weight copy
(host-replicated). The two images' matmuls interleave per tap, so two
row-group streams run concurrently on the PE (2 cols/cycle aggregate).
x arrives in row-bands via SWDGE (gpsimd) so the first matmuls start a
few us in, overlapping the rest of the load; weights/bias load first on
uncontended queues (baseline lost 60us to weight DMAs starved behind
the bulk x transfer). PSUM holds 4 windows x 2 images (8 banks);
windows complete tap-by-tap (w-outer) so the scalar-engine bias+copy
drains each window while the next streams. Output chunks (24 rows) DMA
from staging on the sync HWDGE queue.
"""

import numpy as np
import ml_dtypes
from contextlib import ExitStack

import concourse.bass as bass
import concourse.tile as tile
from concourse import mybir, bacc
from concourse import bass_utils

# Problem shape (hardcoded per contract)
B, CIN, COUT, H, W = 16, 128, 128, 128, 128
KH, KW = 3, 3
BLK = 32
NCORES = 8
BPC = B // NCORES            # images per core

HP, WP = H + 2, W + 2        # host-padded image
LF = HP * WP                 # flat padded image length (16900)
RPW = 4                      # output rows per PSUM window (N=512 = full bank)
NWIN = H // RPW              # 32 uniform windows
CHUNK_WIN = 2                # windows staged per output DMA (8 rows)
NWARM = 6                    # PE warm-up matmuls (ramp DVFS before x lands)
WARM_N = 256                 # columns per warm-up matmul

# x row-bands (in padded rows): first band small for a fast pipeline start.
# Band 0 goes on the scalar HWDGE queue (idle at startup); the rest stream
# on the gpsimd SWDGE queue.
BAND0 = (0, 8)
BANDS = [(8, 24), (32, 40), (72, 40), (112, 18)]

_cache = {}


def _build(K_used, halves):
    """Build + compile the per-core SPMD program.

    K_used: number of surviving input channels (multiple of 32)
    halves: both images packed into one 128-partition buffer at bases
            (0, K_used) for concurrent PE row-group streams
    """
    P = 2 * K_used if halves else K_used
    assert P <= 128

    nc = bacc.Bacc("TRN2", target_bir_lowering=False, debug=False)
    f32 = mybir.dt.float32
    bf16 = mybir.dt.bfloat16

    x_in = nc.dram_tensor("xp", [BPC, K_used, HP, WP], bf16, kind="ExternalInput").ap()
    wt_in = nc.dram_tensor("wt", [P, KH * KW, COUT], bf16, kind="ExternalInput").ap()
    b_in = nc.dram_tensor("bias2d", [COUT, 1], f32, kind="ExternalInput").ap()
    y_out = nc.dram_tensor("y", [BPC, COUT, H, W], f32, kind="ExternalOutput").ap()

    with tile.TileContext(nc) as tc:
        with ExitStack() as ctx:
            singles = ctx.enter_context(tc.tile_pool(name="singles", bufs=1))
            stage_pool = ctx.enter_context(tc.tile_pool(name="ystage", bufs=6))
            psum_pool = ctx.enter_context(
                tc.tile_pool(name="psum", bufs=8, space="PSUM")
            )

            # ---- small loads first: weights+bias on the sync HWDGE queue
            # (uncontended; x bands own the gpsimd SWDGE queue) ------------
            wT = singles.tile([P, KH * KW, COUT], bf16, name="wT")
            nc.sync.dma_start(out=wT, in_=wt_in)

            bias_sb = singles.tile([COUT, 1], f32, name="bias_sb")
            nc.sync.dma_start(out=bias_sb, in_=b_in)

            # ---- PE warm-up: zero matmuls with no DMA dependencies keep
            # the tensor engine busy through the p-state ramp while the
            # first x bands are still in flight --------------------------
            warm = singles.tile([128, 512], bf16, name="warm")
            nc.gpsimd.memset(warm, 0.0)
            warm_ps = psum_pool.tile([128, 512], f32, tag="ps", name="warm_ps")
            for i in range(NWARM):
                nc.tensor.matmul(
                    warm_ps[:, :WARM_N],
                    warm[0:64, 0:128],
                    warm[0:64, :WARM_N],
                    start=True,
                    stop=True,
                )

            # ---- x: flat padded images, loaded in row-bands (SWDGE) -------
            if halves:
                xb0 = singles.tile([128, LF], bf16, name="xbuf")
                xbufs = [xb0, xb0]
                pbase = [0, K_used]
            else:
                xbufs = [
                    singles.tile([K_used, LF], bf16, name=f"xbuf{i}")
                    for i in range(BPC)
                ]
                pbase = [0] * BPC
            r0, nr = BAND0
            for b in range(BPC):
                src = x_in[b].rearrange("c h w -> c (h w)")
                nc.scalar.dma_start(
                    out=xbufs[b][pbase[b] : pbase[b] + K_used, r0 * WP : (r0 + nr) * WP],
                    in_=src[:, r0 * WP : (r0 + nr) * WP],
                )
            for r0, nr in BANDS:
                for b in range(BPC):
                    src = x_in[b].rearrange("c h w -> c (h w)")
                    nc.gpsimd.dma_start(
                        out=xbufs[b][pbase[b] : pbase[b] + K_used, r0 * WP : (r0 + nr) * WP],
                        in_=src[:, r0 * WP : (r0 + nr) * WP],
                    )

            # ---- main loop: windows of RPW output rows --------------------
            for c0 in range(0, NWIN, CHUNK_WIN):
                wins = list(range(c0, min(c0 + CHUNK_WIN, NWIN)))
                chunk_r0 = RPW * c0
                chunk_nr = min(RPW * len(wins), H - chunk_r0)
                stages = [
                    stage_pool.tile(
                        [COUT, RPW * CHUNK_WIN * W], f32, tag=f"st{b}", name=f"st{b}_{c0}"
                    )
                    for b in range(BPC)
                ]

                for w in wins:
                    r0 = RPW * w
                    nrows = min(RPW, H - r0)
                    N = nrows * W
                    ps = [
                        psum_pool.tile([128, 512], f32, tag="ps", name=f"ps{b}_{w}")
                        for b in range(BPC)
                    ]
                    for t in range(KH * KW):
                        dh, dw = divmod(t, 3)
                        off = (r0 + dh) * WP + dw
                        for b in range(BPC):
                            v = xbufs[b][pbase[b] : pbase[b] + K_used, off : off + 1]
                            rhs = bass.AP(
                                tensor=v.tensor,
                                offset=v.offset,
                                ap=[list(v.ap[0]), [WP, nrows], [1, W]],
                            )
                            nc.tensor.matmul(
                                ps[b][:, :N],
                                wT[pbase[b] : pbase[b] + K_used, t, :],
                                rhs,
                                start=(t == 0),
                                stop=(t == KH * KW - 1),
                            )
                    # per-window bias + PSUM->SBUF drain on the scalar engine
                    so = (w - c0) * RPW * W
                    for b in range(BPC):
                        nc.scalar.activation(
                            out=stages[b][:, so : so + N],
                            in_=ps[b][:, :N],
                            func=mybir.ActivationFunctionType.Identity,
                            bias=bias_sb,
                            scale=1.0,
                        )

                for b in range(BPC):
                    nc.sync.dma_start(
                        out=y_out[b][:, chunk_r0 : chunk_r0 + chunk_nr, :],
                        in_=stages[b][:, : chunk_nr * W].rearrange(
                            "p (r s) -> p r s", s=W
                        ),
                    )

    nc.compile()
    return nc


def kernel(x, weight, bias, mask):
    x = np.ascontiguousarray(np.asarray(x, dtype=np.float32))
    weight = np.ascontiguousarray(np.asarray(weight, dtype=np.float32))
    bias = np.ascontiguousarray(np.asarray(bias, dtype=np.float32))
    mask = np.ascontiguousarray(np.asarray(mask, dtype=np.float32))
    bf16 = ml_dtypes.bfloat16

    # --- host-side schedule specialization from the runtime mask ----------
    wm = weight * mask
    blk_any = (
        np.abs(wm).reshape(COUT, CIN // BLK, BLK, KH, KW).sum(axis=(0, 2, 3, 4)) > 0
    )
    used_ibs = [ib for ib in range(CIN // BLK) if blk_any[ib]]
    if not used_ibs:
        used_ibs = [0]
    K_used = BLK * len(used_ibs)
    halves = 2 * K_used <= 128

    used_ch = np.concatenate(
        [np.arange(ib * BLK, (ib + 1) * BLK) for ib in used_ibs]
    )

    key = (K_used, halves)
    if key not in _cache:
        _cache[key] = _build(K_used, halves)
    nc = _cache[key]

    # lhsT layout: wt[c, t, o] = (w*m)[o, used_ch[c], t], replicated per
    # partition half so each image's row group has its own copy
    wt = wm[:, used_ch].reshape(COUT, K_used, KH * KW).transpose(1, 2, 0)
    if halves:
        wt = np.concatenate([wt, wt], axis=0)
    wt = np.ascontiguousarray(wt.astype(bf16))
    bias2d = np.ascontiguousarray(bias[:, None])

    in_maps = []
    for core in range(NCORES):
        xs = x[core * BPC : (core + 1) * BPC][:, used_ch]
        xp = np.zeros((BPC, K_used, HP, WP), dtype=bf16)
        xp[:, :, 1 : H + 1, 1 : W + 1] = xs.astype(bf16)
        in_maps.append({"xp": xp, "wt": wt, "bias2d": bias2d})

    global _last_in_maps
    _last_in_maps = in_maps

    res = bass_utils.run_bass_kernel_spmd(nc, in_maps, core_ids=list(range(NCORES)))
    y = np.concatenate([res.results[c]["y"] for c in range(NCORES)], axis=0)
    return y


_last_in_maps = None
